# revision 26
# baseline (speedup 1.0000x reference)
"""Trainium2 Bass kernel for DeepInterestNetwork (DIN).

8 cores, data-parallel over batch; each core: 512 rows = 16 tiles of G=32.
Host marshals embedding gathers into per-row fp8 payloads; all MLP/softmax/
pooling compute runs on-device.

Per batch row g the attention layer folds to ONE K=65 matmul:
    h_g = [k_g | 1] @ lhsT_g,   lhsT_g = [(Wk-Wd) + diag(q_g) Wm ; c_g]
with c_g = q_g@(Wq+Wd) + b1 built host-side, so the per-row bias rides in a
ones-row of the streamed rhs and the relu drain needs no bias.

Scores are produced TRANSPOSED ([t, row] layout) by using the drained h as
the stationary matmul operand against a block-diagonal w2 — this puts the
softmax free-dim at 64 per partition so exp is one cheap ACT op per tile
instead of 16. Softmax normalization is deferred through pooling: the
pooling matmul (k stationary incl. a ones column, exp-scores streaming)
yields [interest_raw ; denom] per row.

Pipelining: pooling of tile ti is emitted one iteration later (so a late
exp never head-of-line blocks the next tile's attention matmuls in PE's
in-order queue), and the normalize+head MLP runs in column quarters after
every 4 tiles (reciprocal -> rank-1 PE broadcast of 1/denom across
partitions -> multiply -> d1/d2/out matmuls with DVE biased relus),
leaving only the final sigmoid + output DMA as the serial tail.
"""

import numpy as np
import sys

for p in ("/opt/trn_rl_repo", "/opt/trn_rl_repo/concourse"):
    if p not in sys.path:
        sys.path.insert(0, p)

VOCAB, E = 100000, 64
B, T = 4096, 200
NCORES = 8
BC = B // NCORES          # 512 rows per core
G = 32                    # batch rows per tile
NTILES = BC // G          # 16
TC = 100                  # t-chunk for pooling contraction
NC_CH = T // TC           # 2 chunks

_nc_cache = {}


def build_nc(ntiles=NTILES):
    import concourse.bacc as bacc
    import concourse.mybir as mybir
    import concourse.tile as tile

    f32 = mybir.dt.float32
    f16 = mybir.dt.float16
    f8 = mybir.dt.float8e4
    AF = mybir.ActivationFunctionType
    ALU = mybir.AluOpType

    nc = bacc.Bacc("TRN2", target_bir_lowering=False, debug=False)

    # ---- DRAM tensors (inputs) ----
    katt_d = nc.dram_tensor("katt", [ntiles, 65, G, T], f8, kind="ExternalInput")
    lw_d = nc.dram_tensor("lw", [ntiles, 65, G, E], f8, kind="ExternalInput")
    kpool_d = nc.dram_tensor("kpool", [ntiles, TC, NC_CH, G, 65], f8,
                             kind="ExternalInput")
    w2dbl_d = nc.dram_tensor("w2dbl", [128, 2], f16, kind="ExternalInput")
    ones64_d = nc.dram_tensor("ones64", [1, E], f32, kind="ExternalInput")
    dw1_d = nc.dram_tensor("dw1", [E, 128], f16, kind="ExternalInput")
    db1_d = nc.dram_tensor("db1", [128, 1], f32, kind="ExternalInput")
    dw2_d = nc.dram_tensor("dw2", [128, E], f16, kind="ExternalInput")
    db2_d = nc.dram_tensor("db2", [E, 1], f32, kind="ExternalInput")
    ow_d = nc.dram_tensor("ow", [E, 1], f16, kind="ExternalInput")
    ob_d = nc.dram_tensor("ob", [1, 1], f32, kind="ExternalInput")
    out_d = nc.dram_tensor("out", [1, ntiles * G], f32, kind="ExternalOutput")

    with tile.TileContext(nc) as tc:
        with tc.tile_pool(name="consts", bufs=1) as consts, \
             tc.tile_pool(name="kattp", bufs=3) as katt_pool, \
             tc.tile_pool(name="lwp", bufs=3) as lw_pool, \
             tc.tile_pool(name="kpoolp", bufs=3) as kpool_pool, \
             tc.tile_pool(name="hp", bufs=3) as h_pool, \
             tc.tile_pool(name="esp", bufs=3) as es_pool, \
             tc.tile_pool(name="itp", bufs=1) as it_pool, \
             tc.tile_pool(name="smallp", bufs=2) as small_pool, \
             tc.tile_pool(name="ph", bufs=3, space="PSUM") as ph_pool, \
             tc.tile_pool(name="sc", bufs=2, space="PSUM") as sc_pool, \
             tc.tile_pool(name="pool", bufs=3, space="PSUM") as pool_pool:

            # ---- constants ----
            w2dbl = consts.tile([128, 2], f16)
            nc.scalar.dma_start(w2dbl[:], w2dbl_d.ap())
            ones64 = consts.tile([1, E], f32)
            nc.scalar.dma_start(ones64[:], ones64_d.ap())
            dw1 = consts.tile([E, 128], f16)
            nc.scalar.dma_start(dw1[:], dw1_d.ap())
            db1 = consts.tile([128, 1], f32)
            nc.scalar.dma_start(db1[:], db1_d.ap())
            dw2 = consts.tile([128, E], f16)
            nc.scalar.dma_start(dw2[:], dw2_d.ap())
            db2 = consts.tile([E, 1], f32)
            nc.scalar.dma_start(db2[:], db2_d.ap())
            ow = consts.tile([E, 1], f16)
            nc.scalar.dma_start(ow[:], ow_d.ap())
            ob = consts.tile([1, 1], f32)
            nc.scalar.dma_start(ob[:], ob_d.ap())

            itraw = it_pool.tile([65, ntiles * G], f16, tag="itraw")

            # Software-pipelined: pooling of tile ti runs one iteration
            # later so late exp(ti) never head-of-line blocks tile ti+1's
            # attention matmuls in PE's in-order queue. Normalization is
            # deferred entirely to the (batched) head.
            pending = None        # (ti, kpool, esT) awaiting pooling

            def pool_tile(ti, kpool, esT):
                pool_ps = pool_pool.tile([65, G], f32, tag="pool")
                for g in range(G):
                    for c in range(NC_CH):
                        nc.tensor.matmul(
                            pool_ps[:, g:g + 1],
                            kpool[:, c, g, :],
                            esT[:, c * G + g:c * G + g + 1],
                            start=(c == 0),
                            stop=(c == NC_CH - 1),
                        )
                nc.vector.tensor_copy(
                    itraw[:, ti * G:(ti + 1) * G], pool_ps[:])

            logits = it_pool.tile([1, ntiles * G], f32, tag="logits")
            QW = 4 * G               # head processed in column quarters

            def head_quarter(q):
                cols = slice(QW * q, QW * (q + 1))
                # normalize: recip -> rank-1 broadcast -> multiply
                rq = small_pool.tile([1, QW], f32, tag="rq")
                nc.vector.reciprocal(rq[:], itraw[E:E + 1, cols])
                rbps = ph_pool.tile([E, QW], f32, tag="ph")
                nc.tensor.matmul(rbps[:], ones64[:], rq[:])
                itq = small_pool.tile([E, QW], f16, tag="itq")
                nc.vector.tensor_mul(itq[:], itraw[0:E, cols], rbps[:])
                # head MLP (biased relus on DVE via per-partition scalar)
                pd1 = ph_pool.tile([128, QW], f32, tag="ph")
                nc.tensor.matmul(pd1[:], dw1[:], itq[:])
                d1h = h_pool.tile([128, QW], f16, tag="d1h")
                nc.vector.tensor_scalar(
                    d1h[:], pd1[:], db1[:], 0.0, ALU.add, ALU.max)
                pd2 = sc_pool.tile([E, QW], f32, tag="scT")
                nc.tensor.matmul(pd2[:], dw2[:], d1h[:])
                d2h = h_pool.tile([E, QW], f16, tag="d2h")
                nc.vector.tensor_scalar(
                    d2h[:], pd2[:], db2[:], 0.0, ALU.add, ALU.max)
                po = pool_pool.tile([1, QW], f32, tag="pool")
                nc.tensor.matmul(po[:], ow[:], d2h[:])
                nc.vector.tensor_copy(logits[:, cols], po[:])

            for ti in range(ntiles):
                # ---- per-tile loads (big ones split across SP / gpsimd) ----
                katt = katt_pool.tile([65, G, T], f8, tag="katt")
                if ti == 0:
                    # chunked so the first att matmuls start ~2 us sooner
                    for ck in range(4):
                        nc.sync.dma_start(
                            katt[:, 8 * ck:8 * ck + 8, :],
                            katt_d.ap()[ti][:, 8 * ck:8 * ck + 8, :])
                else:
                    nc.sync.dma_start(katt[:], katt_d.ap()[ti])
                lw = lw_pool.tile([65, G, E], f8, tag="lw")
                nc.gpsimd.dma_start(lw[:], lw_d.ap()[ti])
                kpool = kpool_pool.tile([TC, NC_CH, G, 65], f8, tag="kpool")
                nc.gpsimd.dma_start(kpool[:], kpool_d.ap()[ti])

                h16 = h_pool.tile([128, G // 2, T], f16, tag="h16")
                scT = sc_pool.tile([TC, NC_CH, G // 2, 2], f32, tag="scT")

                # ---- attention: per-row K=65 matmuls, pairs in column
                # halves; 2 pairs per PSUM bank; relu-drain alternates
                # DVE / ACT. scoresT matmuls (h16 stationary vs block-diag
                # w2) are interleaved two banks behind so PE never waits
                # on a drain. ----
                def scores_bank(b):
                    for j in (2 * b, 2 * b + 1):
                        for c in range(NC_CH):
                            nc.tensor.matmul(
                                scT[:, c, j, :],
                                h16[:, j, c * TC:(c + 1) * TC],
                                w2dbl[:],
                            )

                for b in range(G // 4):            # bank of rows 4b..4b+3
                    ph = ph_pool.tile([128, 2, T], f32, tag="ph")
                    for jj in range(2):            # pair j = 2b+jj
                        for par in range(2):
                            g = 4 * b + 2 * jj + par
                            nc.tensor.matmul(
                                ph[64 * par:64 * par + 64, jj, :],
                                lw[:, g, :],
                                katt[:, g, :],
                                tile_position=(0, 64 * par),
                            )
                    if b % 2 == 0:
                        nc.vector.tensor_scalar(
                            h16[:, 2 * b:2 * b + 2, :], ph[:],
                            1.0 / 256, 0.0, ALU.mult, ALU.max)
                    else:
                        nc.scalar.activation(
                            h16[:, 2 * b:2 * b + 2, :], ph[:], AF.Relu,
                            scale=1.0 / 256)
                    if b >= 2:
                        scores_bank(b - 2)

                # deferred pooling keeps PE busy while the last drains land
                if pending is not None:
                    pool_tile(*pending)
                scores_bank(G // 4 - 2)
                scores_bank(G // 4 - 1)

                # ---- softmax numerator: one exp over [100, 64] ----
                esT = es_pool.tile([TC, NC_CH * G], f8, tag="esT")
                nc.scalar.activation(
                    esT[:], scT[:].rearrange("p c j q -> p (c j q)"), AF.Exp)

                pending = (ti, kpool, esT)
                if ti % 4 == 3 and ti > 3:
                    head_quarter(ti // 4 - 1)

            pool_tile(*pending)
            head_quarter(ntiles // 4 - 1)

            # ---- final sigmoid + output ----
            outb = small_pool.tile([1, ntiles * G], f32, tag="outb")
            nc.scalar.activation(outb[:], logits[:], AF.Sigmoid, bias=ob[:])
            nc.sync.dma_start(out_d.ap(), outb[:])

    nc.compile()
    return nc


def marshal_inputs(query, keys, emb, att_w1, att_b1, att_w2, att_b2,
                   deep_w1, deep_b1, deep_w2, deep_b2, out_w, out_b,
                   ntiles=NTILES):
    import ml_dtypes
    f8 = ml_dtypes.float8_e4m3

    query = np.asarray(query).astype(np.int64)
    keys = np.asarray(keys).astype(np.int64)
    emb = np.asarray(emb, dtype=np.float32)
    a1 = np.asarray(att_w1, dtype=np.float32)
    Wq, Wk, Wd, Wm = a1[0:64], a1[64:128], a1[128:192], a1[192:256]
    Wpp = Wk - Wd                                   # [64, 64]
    Wqd = Wq + Wd                                   # [64, 64]
    b1 = np.asarray(att_b1, np.float32)             # [64]
    w2 = np.asarray(att_w2, np.float32)[:, 0]       # [64]
    w2dbl = np.zeros((128, 2), np.float16)
    w2dbl[0:64, 0] = w2
    w2dbl[64:128, 1] = w2
    # rank-1 broadcast vector; 1/256 undoes the fp8 k-embedding scaling
    ones64 = np.full((1, E), 1.0 / 256, np.float32)
    dw1 = np.asarray(deep_w1, np.float32).astype(np.float16)
    db1 = np.asarray(deep_b1, np.float32).reshape(128, 1)
    dw2 = np.asarray(deep_w2, np.float32).astype(np.float16)
    db2 = np.asarray(deep_b2, np.float32).reshape(64, 1)
    ow = np.asarray(out_w, np.float32).astype(np.float16)
    ob = np.asarray(out_b, np.float32).reshape(1, 1)

    in_maps = []
    for c in range(NCORES):
        rows = slice(c * BC, c * BC + ntiles * G)
        kt = emb[keys[rows]].reshape(ntiles, G, T, E)       # [nt,G,T,E] f32
        qe = emb[query[rows]].reshape(ntiles, G, E)         # [nt,G,E] f32

        # katt[nt, 65, g, t]: rows 0-63 = 256*k^T (fp8 normal range),
        # row 64 = ones; the x256 is undone by the relu-drain scale.
        katt = np.empty((ntiles, 65, G, T), np.float32)
        katt[:, 0:E] = kt.transpose(0, 3, 1, 2) * 256.0
        katt[:, E] = 1.0

        # lw[nt, 65, g, m]: rows 0-63 = Wpp + q_g (x) Wm, row 64 = 256*c_g
        # (so the bias survives the 1/256 drain scale)
        lw = np.empty((ntiles, 65, G, E), np.float32)
        lw[:, 0:E] = (Wpp[None, :, None, :]
                      + qe.transpose(0, 2, 1)[:, :, :, None] * Wm[:, None, :])
        lw[:, E] = (qe @ Wqd + b1) * 256.0

        # kpool[nt, p, c, g, 65]: [256*k | 1] with t = c*TC + p; the x256
        # is undone by the 1/256 in the rank-1 broadcast vector.
        kpool = np.empty((ntiles, TC, NC_CH, G, 65), np.float32)
        ktc = kt.reshape(ntiles, G, NC_CH, TC, E)
        kpool[:, :, :, :, 0:E] = ktc.transpose(0, 3, 2, 1, 4) * 256.0
        kpool[:, :, :, :, E] = 1.0

        in_maps.append({
            "katt": katt.astype(f8), "lw": lw.astype(f8),
            "kpool": kpool.astype(f8),
            "w2dbl": w2dbl, "ones64": ones64,
            "dw1": dw1, "db1": db1, "dw2": dw2, "db2": db2,
            "ow": ow, "ob": ob,
        })
    return in_maps


def kernel(**inputs) -> np.ndarray:
    from concourse.bass_utils import run_bass_kernel_spmd

    if "full" not in _nc_cache:
        _nc_cache["full"] = build_nc(NTILES)
    nc = _nc_cache["full"]
    in_maps = marshal_inputs(**inputs)
    res = run_bass_kernel_spmd(nc, in_maps, core_ids=list(range(NCORES)))
    outs = [res.results[c]["out"].reshape(-1) for c in range(NCORES)]
    return np.concatenate(outs).reshape(B, 1).astype(np.float32)


if __name__ == "__main__":
    sys.path.insert(0, "/root/problem")
    import reference
    inputs = {k: np.asarray(v) for k, v in reference.setup_inputs().items()}
    expected = np.asarray(reference.reference(**inputs))
    actual = kernel(**inputs)
    err = np.abs(actual - expected).max() / (np.abs(expected).max() + 1e-12)
    print("Relative error:", err)


# revision 27
# speedup vs baseline: 1.0140x; 1.0140x over previous
"""Trainium2 Bass kernel for DeepInterestNetwork (DIN).

8 cores, data-parallel over batch; each core: 512 rows = 16 tiles of G=32.
Host marshals embedding gathers into per-row fp8 payloads; all MLP/softmax/
pooling compute runs on-device.

Per batch row g the attention layer folds to ONE K=65 matmul:
    h_g = [k_g | 1] @ lhsT_g,   lhsT_g = [(Wk-Wd) + diag(q_g) Wm ; c_g]
with c_g = q_g@(Wq+Wd) + b1 built host-side, so the per-row bias rides in a
ones-row of the streamed rhs and the relu drain needs no bias.

Scores are produced TRANSPOSED ([t, row] layout) by using the drained h as
the stationary matmul operand against a block-diagonal w2 — this puts the
softmax free-dim at 64 per partition so exp is one cheap ACT op per tile
instead of 16. Softmax normalization is deferred through pooling: the
pooling matmul (k stationary incl. a ones column, exp-scores streaming)
yields [interest_raw ; denom] per row.

Pipelining: pooling of tile ti is emitted one iteration later (so a late
exp never head-of-line blocks the next tile's attention matmuls in PE's
in-order queue), and the normalize+head MLP runs in column quarters after
every 4 tiles (reciprocal -> rank-1 PE broadcast of 1/denom across
partitions -> multiply -> d1/d2/out matmuls with DVE biased relus),
leaving only the final sigmoid + output DMA as the serial tail.
"""

import numpy as np
import sys

for p in ("/opt/trn_rl_repo", "/opt/trn_rl_repo/concourse"):
    if p not in sys.path:
        sys.path.insert(0, p)

VOCAB, E = 100000, 64
B, T = 4096, 200
NCORES = 8
BC = B // NCORES          # 512 rows per core
G = 32                    # batch rows per tile
NTILES = BC // G          # 16
TC = 100                  # t-chunk for pooling contraction
NC_CH = T // TC           # 2 chunks

_nc_cache = {}


def build_nc(ntiles=NTILES):
    import concourse.bacc as bacc
    import concourse.mybir as mybir
    import concourse.tile as tile

    f32 = mybir.dt.float32
    f16 = mybir.dt.float16
    f8 = mybir.dt.float8e4
    AF = mybir.ActivationFunctionType
    ALU = mybir.AluOpType

    nc = bacc.Bacc("TRN2", target_bir_lowering=False, debug=False)

    # ---- DRAM tensors (inputs) ----
    katt_d = nc.dram_tensor("katt", [ntiles, 65, G, T], f8, kind="ExternalInput")
    lw_d = nc.dram_tensor("lw", [ntiles, 65, G, E], f8, kind="ExternalInput")
    kpool_d = nc.dram_tensor("kpool", [ntiles, TC, NC_CH, G, 65], f8,
                             kind="ExternalInput")
    w2dbl_d = nc.dram_tensor("w2dbl", [128, 2], f16, kind="ExternalInput")
    ones64_d = nc.dram_tensor("ones64", [1, E], f32, kind="ExternalInput")
    dw1_d = nc.dram_tensor("dw1", [E, 128], f16, kind="ExternalInput")
    db1_d = nc.dram_tensor("db1", [128, 1], f32, kind="ExternalInput")
    dw2_d = nc.dram_tensor("dw2", [128, E], f16, kind="ExternalInput")
    db2_d = nc.dram_tensor("db2", [E, 1], f32, kind="ExternalInput")
    ow_d = nc.dram_tensor("ow", [E, 1], f16, kind="ExternalInput")
    ob_d = nc.dram_tensor("ob", [1, 1], f32, kind="ExternalInput")
    out_d = nc.dram_tensor("out", [1, ntiles * G], f32, kind="ExternalOutput")

    with tile.TileContext(nc) as tc:
        with tc.tile_pool(name="consts", bufs=1) as consts, \
             tc.tile_pool(name="kattp", bufs=3) as katt_pool, \
             tc.tile_pool(name="lwp", bufs=3) as lw_pool, \
             tc.tile_pool(name="kpoolp", bufs=3) as kpool_pool, \
             tc.tile_pool(name="hp", bufs=3) as h_pool, \
             tc.tile_pool(name="esp", bufs=3) as es_pool, \
             tc.tile_pool(name="itp", bufs=1) as it_pool, \
             tc.tile_pool(name="smallp", bufs=2) as small_pool, \
             tc.tile_pool(name="ph", bufs=4, space="PSUM") as ph_pool, \
             tc.tile_pool(name="sc", bufs=1, space="PSUM") as sc_pool, \
             tc.tile_pool(name="pool", bufs=3, space="PSUM") as pool_pool:

            # ---- constants ----
            w2dbl = consts.tile([128, 2], f16)
            nc.scalar.dma_start(w2dbl[:], w2dbl_d.ap())
            ones64 = consts.tile([1, E], f32)
            nc.scalar.dma_start(ones64[:], ones64_d.ap())
            dw1 = consts.tile([E, 128], f16)
            nc.scalar.dma_start(dw1[:], dw1_d.ap())
            db1 = consts.tile([128, 1], f32)
            nc.scalar.dma_start(db1[:], db1_d.ap())
            dw2 = consts.tile([128, E], f16)
            nc.scalar.dma_start(dw2[:], dw2_d.ap())
            db2 = consts.tile([E, 1], f32)
            nc.scalar.dma_start(db2[:], db2_d.ap())
            ow = consts.tile([E, 1], f16)
            nc.scalar.dma_start(ow[:], ow_d.ap())
            ob = consts.tile([1, 1], f32)
            nc.scalar.dma_start(ob[:], ob_d.ap())

            itraw = it_pool.tile([65, ntiles * G], f16, tag="itraw")

            # Software-pipelined: pooling of tile ti runs one iteration
            # later so late exp(ti) never head-of-line blocks tile ti+1's
            # attention matmuls in PE's in-order queue. Normalization is
            # deferred entirely to the (batched) head.
            pending = None        # (ti, kpool, esT) awaiting pooling

            def pool_tile(ti, kpool, esT):
                pool_ps = pool_pool.tile([65, G], f32, tag="pool")
                for g in range(G):
                    for c in range(NC_CH):
                        nc.tensor.matmul(
                            pool_ps[:, g:g + 1],
                            kpool[:, c, g, :],
                            esT[:, c * G + g:c * G + g + 1],
                            start=(c == 0),
                            stop=(c == NC_CH - 1),
                        )
                nc.vector.tensor_copy(
                    itraw[:, ti * G:(ti + 1) * G], pool_ps[:])

            logits = it_pool.tile([1, ntiles * G], f32, tag="logits")
            QW = 4 * G               # head processed in column quarters

            def head_quarter(q):
                cols = slice(QW * q, QW * (q + 1))
                # normalize: recip -> rank-1 broadcast -> multiply
                rq = small_pool.tile([1, QW], f32, tag="rq")
                nc.vector.reciprocal(rq[:], itraw[E:E + 1, cols])
                rbps = ph_pool.tile([E, QW], f32, tag="ph")
                nc.tensor.matmul(rbps[:], ones64[:], rq[:])
                itq = small_pool.tile([E, QW], f16, tag="itq")
                nc.vector.tensor_mul(itq[:], itraw[0:E, cols], rbps[:])
                # head MLP (biased relus on DVE via per-partition scalar)
                pd1 = ph_pool.tile([128, QW], f32, tag="ph")
                nc.tensor.matmul(pd1[:], dw1[:], itq[:])
                d1h = h_pool.tile([128, QW], f16, tag="d1h")
                nc.vector.tensor_scalar(
                    d1h[:], pd1[:], db1[:], 0.0, ALU.add, ALU.max)
                pd2 = sc_pool.tile([E, QW], f32, tag="scT")
                nc.tensor.matmul(pd2[:], dw2[:], d1h[:])
                d2h = h_pool.tile([E, QW], f16, tag="d2h")
                nc.vector.tensor_scalar(
                    d2h[:], pd2[:], db2[:], 0.0, ALU.add, ALU.max)
                po = pool_pool.tile([1, QW], f32, tag="pool")
                nc.tensor.matmul(po[:], ow[:], d2h[:])
                nc.vector.tensor_copy(logits[:, cols], po[:])

            for ti in range(ntiles):
                # ---- per-tile loads (big ones split across SP / gpsimd) ----
                katt = katt_pool.tile([65, G, T], f8, tag="katt")
                if ti == 0:
                    # chunked so the first att matmuls start ~2 us sooner
                    for ck in range(4):
                        nc.sync.dma_start(
                            katt[:, 8 * ck:8 * ck + 8, :],
                            katt_d.ap()[ti][:, 8 * ck:8 * ck + 8, :])
                else:
                    nc.sync.dma_start(katt[:], katt_d.ap()[ti])
                lw = lw_pool.tile([65, G, E], f8, tag="lw")
                nc.gpsimd.dma_start(lw[:], lw_d.ap()[ti])
                kpool = kpool_pool.tile([TC, NC_CH, G, 65], f8, tag="kpool")
                nc.gpsimd.dma_start(kpool[:], kpool_d.ap()[ti])

                h16 = h_pool.tile([128, G // 2, T], f16, tag="h16")
                scT = sc_pool.tile([TC, NC_CH, G // 2, 2], f32, tag="scT")

                # ---- attention: per-row K=65 matmuls, pairs in column
                # halves; 2 pairs per PSUM bank; relu-drain alternates
                # DVE / ACT. scoresT matmuls (h16 stationary vs block-diag
                # w2) are interleaved two banks behind so PE never waits
                # on a drain. ----
                def scores_bank(b):
                    for j in (2 * b, 2 * b + 1):
                        for c in range(NC_CH):
                            nc.tensor.matmul(
                                scT[:, c, j, :],
                                h16[:, j, c * TC:(c + 1) * TC],
                                w2dbl[:],
                            )

                for b in range(G // 4):            # bank of rows 4b..4b+3
                    ph = ph_pool.tile([128, 2, T], f32, tag="ph")
                    for jj in range(2):            # pair j = 2b+jj
                        for par in range(2):
                            g = 4 * b + 2 * jj + par
                            nc.tensor.matmul(
                                ph[64 * par:64 * par + 64, jj, :],
                                lw[:, g, :],
                                katt[:, g, :],
                                tile_position=(0, 64 * par),
                            )
                    if b % 2 == 0:
                        nc.vector.tensor_scalar(
                            h16[:, 2 * b:2 * b + 2, :], ph[:],
                            1.0 / 256, 0.0, ALU.mult, ALU.max)
                    else:
                        nc.scalar.activation(
                            h16[:, 2 * b:2 * b + 2, :], ph[:], AF.Relu,
                            scale=1.0 / 256)
                    if b >= 2:
                        scores_bank(b - 2)

                # deferred pooling keeps PE busy while the last drains land
                if pending is not None:
                    pool_tile(*pending)
                scores_bank(G // 4 - 2)
                scores_bank(G // 4 - 1)

                # ---- softmax numerator: one exp over [100, 64] ----
                esT = es_pool.tile([TC, NC_CH * G], f8, tag="esT")
                nc.scalar.activation(
                    esT[:], scT[:].rearrange("p c j q -> p (c j q)"), AF.Exp)

                pending = (ti, kpool, esT)
                if ti % 4 == 3 and ti > 3:
                    head_quarter(ti // 4 - 1)

            pool_tile(*pending)
            head_quarter(ntiles // 4 - 1)

            # ---- final sigmoid + output ----
            outb = small_pool.tile([1, ntiles * G], f32, tag="outb")
            nc.scalar.activation(outb[:], logits[:], AF.Sigmoid, bias=ob[:])
            nc.sync.dma_start(out_d.ap(), outb[:])

    nc.compile()
    return nc


def marshal_inputs(query, keys, emb, att_w1, att_b1, att_w2, att_b2,
                   deep_w1, deep_b1, deep_w2, deep_b2, out_w, out_b,
                   ntiles=NTILES):
    import ml_dtypes
    f8 = ml_dtypes.float8_e4m3

    query = np.asarray(query).astype(np.int64)
    keys = np.asarray(keys).astype(np.int64)
    emb = np.asarray(emb, dtype=np.float32)
    a1 = np.asarray(att_w1, dtype=np.float32)
    Wq, Wk, Wd, Wm = a1[0:64], a1[64:128], a1[128:192], a1[192:256]
    Wpp = Wk - Wd                                   # [64, 64]
    Wqd = Wq + Wd                                   # [64, 64]
    b1 = np.asarray(att_b1, np.float32)             # [64]
    w2 = np.asarray(att_w2, np.float32)[:, 0]       # [64]
    w2dbl = np.zeros((128, 2), np.float16)
    w2dbl[0:64, 0] = w2
    w2dbl[64:128, 1] = w2
    # rank-1 broadcast vector; 1/256 undoes the fp8 k-embedding scaling
    ones64 = np.full((1, E), 1.0 / 256, np.float32)
    dw1 = np.asarray(deep_w1, np.float32).astype(np.float16)
    db1 = np.asarray(deep_b1, np.float32).reshape(128, 1)
    dw2 = np.asarray(deep_w2, np.float32).astype(np.float16)
    db2 = np.asarray(deep_b2, np.float32).reshape(64, 1)
    ow = np.asarray(out_w, np.float32).astype(np.float16)
    ob = np.asarray(out_b, np.float32).reshape(1, 1)

    in_maps = []
    for c in range(NCORES):
        rows = slice(c * BC, c * BC + ntiles * G)
        kt = emb[keys[rows]].reshape(ntiles, G, T, E)       # [nt,G,T,E] f32
        qe = emb[query[rows]].reshape(ntiles, G, E)         # [nt,G,E] f32

        # katt[nt, 65, g, t]: rows 0-63 = 256*k^T (fp8 normal range),
        # row 64 = ones; the x256 is undone by the relu-drain scale.
        katt = np.empty((ntiles, 65, G, T), np.float32)
        katt[:, 0:E] = kt.transpose(0, 3, 1, 2) * 256.0
        katt[:, E] = 1.0

        # lw[nt, 65, g, m]: rows 0-63 = Wpp + q_g (x) Wm, row 64 = 256*c_g
        # (so the bias survives the 1/256 drain scale)
        lw = np.empty((ntiles, 65, G, E), np.float32)
        lw[:, 0:E] = (Wpp[None, :, None, :]
                      + qe.transpose(0, 2, 1)[:, :, :, None] * Wm[:, None, :])
        lw[:, E] = (qe @ Wqd + b1) * 256.0

        # kpool[nt, p, c, g, 65]: [256*k | 1] with t = c*TC + p; the x256
        # is undone by the 1/256 in the rank-1 broadcast vector.
        kpool = np.empty((ntiles, TC, NC_CH, G, 65), np.float32)
        ktc = kt.reshape(ntiles, G, NC_CH, TC, E)
        kpool[:, :, :, :, 0:E] = ktc.transpose(0, 3, 2, 1, 4) * 256.0
        kpool[:, :, :, :, E] = 1.0

        in_maps.append({
            "katt": katt.astype(f8), "lw": lw.astype(f8),
            "kpool": kpool.astype(f8),
            "w2dbl": w2dbl, "ones64": ones64,
            "dw1": dw1, "db1": db1, "dw2": dw2, "db2": db2,
            "ow": ow, "ob": ob,
        })
    return in_maps


def kernel(**inputs) -> np.ndarray:
    from concourse.bass_utils import run_bass_kernel_spmd

    if "full" not in _nc_cache:
        _nc_cache["full"] = build_nc(NTILES)
    nc = _nc_cache["full"]
    in_maps = marshal_inputs(**inputs)
    res = run_bass_kernel_spmd(nc, in_maps, core_ids=list(range(NCORES)))
    outs = [res.results[c]["out"].reshape(-1) for c in range(NCORES)]
    return np.concatenate(outs).reshape(B, 1).astype(np.float32)


if __name__ == "__main__":
    sys.path.insert(0, "/root/problem")
    import reference
    inputs = {k: np.asarray(v) for k, v in reference.setup_inputs().items()}
    expected = np.asarray(reference.reference(**inputs))
    actual = kernel(**inputs)
    err = np.abs(actual - expected).max() / (np.abs(expected).max() + 1e-12)
    print("Relative error:", err)
